# revision 38
# baseline (speedup 1.0000x reference)
"""Trainium2 Bass kernel for nn_DecoderBlock (B=8, S=1024, D=256, H=4 heads
of full width 256, FF=1024).

Strategy: pure data parallelism — B=8 batch elements across 8 NeuronCores,
zero collectives. Per core, one full decoder block in "transposed" activation
layout (features on SBUF partitions, tokens on the free dim) so every matmul
chains without transposes:

  qT/kT = (wT)^T @ xT       (per head, [E, S])
  v     = (xT)^T @ wvT      (natural [S, E] — the att@v stationary operand)
  expT  = exp((kT^T qT)/16) (causal lower-triangle only, [t, s] tiles)
  Z     = ones^T @ expT     (softmax denominators; the ones[128,128] operand
                             leaves Z replicated on every partition, so the
                             reciprocal and normalize run partition-parallel)
  oT    = v^T @ expT        (unnormalized head outputs, normalized by 1/Z)
  mhT   = woT^T @ oT ; LN1 ; ffn (ff1 relu ff2) ; LN2   (all transposed)

LayerNorm means/mean-squares come from ones(1/D) matmuls, replicated across
partitions the same way. Weights are pre-transposed and cast to bf16 on the
host; matmuls run bf16 (4x the fp32 PE rate), accumulation and LN math in
fp32. Dummy warmup matmuls at kernel start keep the PE HAM clock-gate at
full rate while the input DMAs land.

The attention_mask input is all ones per the problem spec (causal mask only);
if a mask with zeros ever shows up, we fall back to a numpy reference.
"""

import numpy as np
import ml_dtypes

import concourse.bass as bass
import concourse.mybir as mybir
import concourse.tile as tile
from concourse import bacc
from concourse.bass_utils import run_bass_kernel_spmd

F32 = mybir.dt.float32
BF16 = mybir.dt.bfloat16
AF = mybir.ActivationFunctionType
ALU = mybir.AluOpType

N_CORES = 8
B, S, D, H, E, HE, FF = 8, 1024, 256, 4, 256, 1024, 1024
SC = 512          # token (free-dim) chunk
NJ = S // SC      # 2 chunks of tokens
ND = D // 128     # 2 partition chunks of features
NF = FF // 128    # 8 partition chunks of ff features
NT = S // 128     # 8 partition chunks of tokens
LN_EPS = 1e-5
SCALE = 1.0 / 16.0  # 1/sqrt(D)

_CACHE = {}


def _build():
    nc = bacc.Bacc("TRN2", target_bir_lowering=False, debug=False,
                   num_devices=N_CORES)

    # ---- DRAM parameters (per-core shard + replicated weights) ----
    xT_d = nc.dram_tensor("xT", [ND, 128, S], F32, kind="ExternalInput")
    xTb_d = nc.dram_tensor("xTb", [ND, 128, S], BF16, kind="ExternalInput")
    wqT_d = nc.dram_tensor("wqT", [ND, 128, HE], BF16, kind="ExternalInput")
    wkT_d = nc.dram_tensor("wkT", [ND, 128, HE], BF16, kind="ExternalInput")
    wvT_d = nc.dram_tensor("wvT", [ND, 128, HE], BF16, kind="ExternalInput")
    woT_d = nc.dram_tensor("woT", [NT, 128, D], BF16, kind="ExternalInput")
    ff1T_d = nc.dram_tensor("ff1T", [ND, 128, FF], BF16, kind="ExternalInput")
    ff2T_d = nc.dram_tensor("ff2T", [NF, 128, D], BF16, kind="ExternalInput")
    wo_b_d = nc.dram_tensor("wo_b", [ND, 128, 1], F32, kind="ExternalInput")
    ff1_b_d = nc.dram_tensor("ff1_b", [NF, 128, 1], F32, kind="ExternalInput")
    ff2_b_d = nc.dram_tensor("ff2_b", [ND, 128, 1], F32, kind="ExternalInput")
    ln1_g_d = nc.dram_tensor("ln1_g", [ND, 128, 1], F32, kind="ExternalInput")
    ln1_b_d = nc.dram_tensor("ln1_b", [ND, 128, 1], F32, kind="ExternalInput")
    ln2_g_d = nc.dram_tensor("ln2_g", [ND, 128, 1], F32, kind="ExternalInput")
    ln2_b_d = nc.dram_tensor("ln2_b", [ND, 128, 1], F32, kind="ExternalInput")
    out_d = nc.dram_tensor("out", [ND, 128, S], F32, kind="ExternalOutput")

    with tile.TileContext(nc) as tc:
        with tc.tile_pool(name="consts", bufs=1) as consts, \
             tc.tile_pool(name="acts", bufs=1) as acts, \
             tc.tile_pool(name="work", bufs=2) as work, \
             tc.tile_pool(name="lnp", bufs=1) as lnp, \
             tc.tile_pool(name="psA", bufs=3, space="PSUM") as psA, \
             tc.tile_pool(name="psO", bufs=3, space="PSUM") as psO, \
             tc.tile_pool(name="psZ", bufs=2, space="PSUM") as psZ:

            # ---- constants / weights into SBUF ----
            dma_engines = [nc.sync]
            dma_rr = [0]

            def load2(dram, shape, dt, n):
                ts_ = []
                for i in range(n):
                    t = consts.tile(shape, dt, tag=f"{dram.name}{i}", name=f"{dram.name}{i}")
                    eng = dma_engines[dma_rr[0] % len(dma_engines)]
                    dma_rr[0] += 1
                    eng.dma_start(out=t[:], in_=dram[i])
                    ts_.append(t)
                return ts_

            # DMA in first-use order so the PE can start ASAP
            xTb = load2(xTb_d, [128, S], BF16, ND)
            wqT = load2(wqT_d, [128, HE], BF16, ND)
            wkT = load2(wkT_d, [128, HE], BF16, ND)
            wvT = load2(wvT_d, [128, HE], BF16, ND)
            xT = load2(xT_d, [128, S], F32, ND)
            woT = load2(woT_d, [128, D], BF16, NT)
            ff1T = load2(ff1T_d, [128, FF], BF16, ND)
            ff2T = load2(ff2T_d, [128, D], BF16, NF)
            wo_b = load2(wo_b_d, [128, 1], F32, ND)
            ff1_b = load2(ff1_b_d, [128, 1], F32, NF)
            ff2_b = load2(ff2_b_d, [128, 1], F32, ND)
            ln1_g = load2(ln1_g_d, [128, 1], F32, ND)
            ln1_b = load2(ln1_b_d, [128, 1], F32, ND)
            ln2_g = load2(ln2_g_d, [128, 1], F32, ND)
            ln2_b = load2(ln2_b_d, [128, 1], F32, ND)

            # "ones" matrices for partition-dim reductions via matmul; the
            # [128,128] shape leaves the sum REPLICATED on all partitions so
            # downstream row math runs partition-parallel with no broadcast.
            ones_bf = consts.tile([128, 128], BF16, tag="ones", name="ones")
            nc.vector.memset(ones_bf[:], 1.0)
            invd_bf = consts.tile([128, 128], BF16, tag="invd", name="invd")
            nc.vector.memset(invd_bf[:], 1.0 / D)  # 2^-8, exact in bf16
            eps_t = consts.tile([128, 1], F32, tag="eps", name="eps")
            nc.vector.memset(eps_t[:], LN_EPS)
            # multiplicative causal mask for the diagonal 128x128 block of a
            # transposed [t, s] exp tile: 1 where t <= s else 0
            cmaskf = consts.tile([128, 128], F32, tag="cmaskf", name="cmaskf")
            nc.gpsimd.memset(cmaskf[:], 1.0)
            nc.gpsimd.affine_select(
                out=cmaskf[:], in_=cmaskf[:],
                compare_op=ALU.is_ge, fill=0.0,
                base=0, pattern=[[1, 128]], channel_multiplier=-1,
            )
            cmask = consts.tile([128, 128], BF16, tag="cmask", name="cmask")
            nc.vector.tensor_copy(out=cmask[:], in_=cmaskf[:])

            # PE warmup: ~4us of dummy matmuls (no DMA dependency) so the
            # HAM clock gate reaches 8/8 while the input DMAs land; real
            # matmuls then start at full clock.
            for wi in range(32):
                wp = psA.tile([128, 128], F32, tag="mm", name="warm")
                nc.tensor.matmul(wp[:], ones_bf[:], ones_bf[:],
                                 start=True, stop=True)

            # ---- attention: per head -> ONT [HE, S] normalized heads (bf16)
            ont = [acts.tile([128, S], BF16, tag=f"ont{c}", name=f"ont{c}") for c in range(NT)]

            for h in range(H):
                # Q^T, K^T [E, S] (transposed), V [S, E] (natural), bf16
                qh = [work.tile([128, S], BF16, tag=f"qh{e0}", name=f"qh{e0}") for e0 in range(2)]
                kh = [work.tile([128, S], BF16, tag=f"kh{e0}", name=f"kh{e0}") for e0 in range(2)]
                vh = [work.tile([128, E], BF16, tag=f"vh{t0}", name=f"vh{t0}") for t0 in range(NT)]
                for e0 in range(2):
                    for j in range(NJ):
                        cols = slice(j * SC, (j + 1) * SC)
                        qp = psA.tile([128, SC], F32, tag="mm", name="mm")
                        for d0 in range(ND):
                            nc.tensor.matmul(
                                qp[:], wqT[d0][:, h * E + e0 * 128: h * E + (e0 + 1) * 128],
                                xTb[d0][:, cols], start=(d0 == 0), stop=(d0 == ND - 1))
                        nc.scalar.copy(out=qh[e0][:, cols], in_=qp[:])
                        kp = psA.tile([128, SC], F32, tag="mm", name="mm")
                        for d0 in range(ND):
                            nc.tensor.matmul(
                                kp[:], wkT[d0][:, h * E + e0 * 128: h * E + (e0 + 1) * 128],
                                xTb[d0][:, cols], start=(d0 == 0), stop=(d0 == ND - 1))
                        nc.vector.tensor_copy(out=kh[e0][:, cols], in_=kp[:])
                for t0 in range(NT):
                    vp = psA.tile([128, E], F32, tag="mm", name="mm")
                    for d0 in range(ND):
                        nc.tensor.matmul(
                            vp[:], xTb[d0][:, t0 * 128:(t0 + 1) * 128],
                            wvT[d0][:, h * E:(h + 1) * E],
                            start=(d0 == 0), stop=(d0 == ND - 1))
                    nc.vector.tensor_copy(out=vh[t0][:], in_=vp[:])

                zp = [psZ.tile([128, SC], F32, tag="z", name="z")
                      for j in range(NJ)]
                op = [[psO.tile([128, SC], F32, tag="o", name="o")
                       for _ in range(2)] for j in range(NJ)]
                for j in range(NJ):
                    kmax = 4 * j + 4
                    pend = []

                    def emit_zo(item):
                        jj, kk, ek, off, w = item
                        km = 4 * jj + 4
                        nc.tensor.matmul(
                            zp[jj][:, off:off + w], ones_bf[:],
                            ek[:, off:off + w],
                            start=(kk == 0), stop=(kk == km - 1),
                            skip_group_check=True)
                        for e0 in range(2):
                            nc.tensor.matmul(
                                op[jj][e0][:, off:off + w],
                                vh[kk][:, e0 * 128:(e0 + 1) * 128],
                                ek[:, off:off + w],
                                start=(kk == 0), stop=(kk == km - 1),
                                skip_group_check=True)

                    for k in range(kmax):
                        start_col = max(SC * j, 128 * k)
                        off = start_col - SC * j
                        w = SC - off
                        sp = psA.tile([128, SC], F32, tag="mm", name="mm")
                        for e0 in range(2):
                            nc.tensor.matmul(
                                sp[:, off:off + w],
                                kh[e0][:, 128 * k:128 * (k + 1)],
                                qh[e0][:, start_col:start_col + w],
                                start=(e0 == 0), stop=(e0 == 1))
                        ek = work.tile([128, SC], BF16, tag=f"exp{k}",
                                       name=f"exp{k}")
                        nc.scalar.activation(
                            out=ek[:, off:off + w], in_=sp[:, off:off + w],
                            func=AF.Exp, scale=SCALE)
                        if 128 * k >= SC * j:  # diagonal band: causal mask
                            nc.vector.tensor_mul(
                                out=ek[:, off:off + 128],
                                in0=ek[:, off:off + 128], in1=cmask[:])
                        pend.append((j, k, ek, off, w))
                        if len(pend) > 2:
                            emit_zo(pend.pop(0))
                    for item in pend:
                        emit_zo(item)
                for j in range(NJ):
                    # normalize: ONT = oT * (1/Z); Z is replicated on all
                    # partitions by the ones[128,128] matmul
                    zb = work.tile([128, SC], F32, tag="zb", name="zb")
                    nc.vector.reciprocal_approx_fast(out=zb[:], in_=zp[j][:])
                    cols = slice(j * SC, (j + 1) * SC)
                    for e0 in range(2):
                        nc.vector.tensor_mul(
                            out=ont[h * 2 + e0][:, cols], in0=op[j][e0][:],
                            in1=zb[:])

            # ---- wo projection + residual -> R1 (f32, transposed) ----
            r1 = [acts.tile([128, S], F32, tag=f"r1_{d0}", name=f"r1_{d0}") for d0 in range(ND)]
            r1b = [lnp.tile([128, S], BF16, tag=f"lnsrcb{d0}", name=f"r1b_{d0}") for d0 in range(ND)]
            for d0 in range(ND):
                for j in range(NJ):
                    cols = slice(j * SC, (j + 1) * SC)
                    pp = psA.tile([128, SC], F32, tag="mm", name="mm")
                    for c in range(NT):
                        nc.tensor.matmul(
                            pp[:], woT[c][:, d0 * 128:(d0 + 1) * 128],
                            ont[c][:, cols], start=(c == 0), stop=(c == NT - 1))
                    # R1 = psum + wo_b + x (f32 + bf16 twin for LN stats)
                    nc.vector.scalar_tensor_tensor(
                        out=r1[d0][:, cols], in0=pp[:], scalar=wo_b[d0][:],
                        in1=xT[d0][:, cols], op0=ALU.add, op1=ALU.add)
                    nc.vector.scalar_tensor_tensor(
                        out=r1b[d0][:, cols], in0=pp[:], scalar=wo_b[d0][:],
                        in1=xT[d0][:, cols], op0=ALU.add, op1=ALU.add)

            def layernorm(src, srcb, gamma, beta, dst, tag, dma_out=None,
                          dst_bf=None):
                """src/srcb: ND x [128, S] f32/bf16 tiles (features on
                partitions). dst: ND x [128, S] f32; dst_bf: optional bf16
                copies; dma_out: optional DRAM targets."""
                sq = [lnp.tile([128, S], BF16, tag=f"lnsq{d0}", name=f"{tag}sq{d0}")
                      for d0 in range(ND)]
                for d0 in range(ND):
                    nc.vector.tensor_mul(out=sq[d0][:], in0=srcb[d0][:],
                                         in1=srcb[d0][:])
                for j in range(NJ):
                    cols = slice(j * SC, (j + 1) * SC)
                    # mean / mean-of-squares directly (ones are 1/D),
                    # replicated on all 128 partitions
                    mup = psZ.tile([128, SC], F32, tag="z", name="z")
                    for d0 in range(ND):
                        nc.tensor.matmul(mup[:], invd_bf[:],
                                         srcb[d0][:, cols],
                                         start=(d0 == 0), stop=(d0 == ND - 1))
                    m2p = psZ.tile([128, SC], F32, tag="z", name="z")
                    for d0 in range(ND):
                        nc.tensor.matmul(m2p[:], invd_bf[:],
                                         sq[d0][:, cols],
                                         start=(d0 == 0), stop=(d0 == ND - 1))
                    musq = work.tile([128, SC], F32, tag="musq", name="musq")
                    nc.scalar.activation(out=musq[:], in_=mup[:],
                                         func=AF.Square)
                    var = work.tile([128, SC], F32, tag="var", name="var")
                    nc.vector.tensor_sub(out=var[:], in0=m2p[:], in1=musq[:])
                    sd = work.tile([128, SC], F32, tag="sd", name="sd")
                    nc.scalar.activation(out=sd[:], in_=var[:], func=AF.Sqrt,
                                         bias=eps_t[:])
                    rstd = work.tile([128, SC], F32, tag="rstd", name="rstd")
                    nc.vector.reciprocal_approx_fast(out=rstd[:], in_=sd[:])
                    mr = work.tile([128, SC], F32, tag="mr", name="mr")
                    nc.vector.tensor_mul(out=mr[:], in0=mup[:], in1=rstd[:])
                    for d0 in range(ND):
                        t = work.tile([128, SC], F32, tag="lnt", name="lnt")
                        nc.vector.tensor_mul(out=t[:], in0=src[d0][:, cols],
                                             in1=rstd[:])
                        nc.vector.tensor_sub(out=t[:], in0=t[:], in1=mr[:])
                        nc.vector.tensor_scalar(
                            out=dst[d0][:, cols], in0=t[:],
                            scalar1=gamma[d0][:], scalar2=beta[d0][:],
                            op0=ALU.mult, op1=ALU.add)
                        if dst_bf is not None:
                            nc.scalar.copy(out=dst_bf[d0][:, cols],
                                           in_=dst[d0][:, cols])
                        if dma_out is not None:
                            nc.sync.dma_start(out=dma_out[d0][:, cols],
                                              in_=dst[d0][:, cols])

            # ---- LN1 -> x1 (f32) + bf16 copy for the FFN ----
            x1 = [acts.tile([128, S], F32, tag=f"x1_{d0}", name=f"x1_{d0}") for d0 in range(ND)]
            x1b = [acts.tile([128, S], BF16, tag=f"x1b{d0}", name=f"x1b{d0}") for d0 in range(ND)]
            layernorm(r1, r1b, ln1_g, ln1_b, x1, "ln1", dst_bf=x1b)

            # ---- FFN ----
            hT = [acts.tile([128, S], BF16, tag=f"hT{f0}", name=f"hT{f0}") for f0 in range(NF)]
            for f0 in range(NF):
                for j in range(NJ):
                    cols = slice(j * SC, (j + 1) * SC)
                    fp = psA.tile([128, SC], F32, tag="mm", name="mm")
                    for d0 in range(ND):
                        nc.tensor.matmul(
                            fp[:], ff1T[d0][:, f0 * 128:(f0 + 1) * 128],
                            x1b[d0][:, cols], start=(d0 == 0), stop=(d0 == ND - 1))
                    # relu(psum + b) fused on ACT
                    nc.scalar.activation(out=hT[f0][:, cols], in_=fp[:],
                                         func=AF.Relu, bias=ff1_b[f0][:])
            r2 = [acts.tile([128, S], F32, tag=f"r2_{d0}", name=f"r2_{d0}") for d0 in range(ND)]
            r2b = [lnp.tile([128, S], BF16, tag=f"lnsrcb{d0}", name=f"r2b_{d0}") for d0 in range(ND)]
            for d0 in range(ND):
                for j in range(NJ):
                    cols = slice(j * SC, (j + 1) * SC)
                    fp = psA.tile([128, SC], F32, tag="mm", name="mm")
                    for f0 in range(NF):
                        nc.tensor.matmul(
                            fp[:], ff2T[f0][:, d0 * 128:(d0 + 1) * 128],
                            hT[f0][:, cols], start=(f0 == 0), stop=(f0 == NF - 1))
                    nc.vector.scalar_tensor_tensor(
                        out=r2[d0][:, cols], in0=fp[:], scalar=ff2_b[d0][:],
                        in1=x1[d0][:, cols], op0=ALU.add, op1=ALU.add)
                    nc.vector.scalar_tensor_tensor(
                        out=r2b[d0][:, cols], in0=fp[:], scalar=ff2_b[d0][:],
                        in1=x1[d0][:, cols], op0=ALU.add, op1=ALU.add)

            # ---- LN2 -> out ----
            outT = [acts.tile([128, S], F32, tag=f"out{d0}", name=f"out{d0}") for d0 in range(ND)]
            layernorm(r2, r2b, ln2_g, ln2_b, outT, "ln2",
                      dma_out=[out_d[d0] for d0 in range(ND)])

    nc.compile()
    return nc


def _np_reference(x, attention_mask, wq, wk, wv, wo_w, wo_b, ln1_g, ln1_b,
                  ff1_w, ff1_b, ff2_w, ff2_b, ln2_g, ln2_b):
    """Numpy fallback (only used if attention_mask has zeros)."""
    def ln(t, g, b):
        mu = t.mean(-1, keepdims=True)
        var = t.var(-1, keepdims=True)
        return (t - mu) / np.sqrt(var + LN_EPS) * g + b
    Bn, Sn, Dn = x.shape
    q = np.einsum('bsd,hed->bhse', x, wq)
    k = np.einsum('bsd,hed->bhse', x, wk)
    v = np.einsum('bsd,hed->bhse', x, wv)
    sc = np.einsum('bhse,bhte->bhst', q, k) / np.sqrt(np.float32(Dn))
    idx = np.arange(Sn)
    causal = idx[None, :] > idx[:, None]
    m = attention_mask.astype(bool)
    valid = m[:, None, :] & m[:, :, None]
    cond = causal[None] | ~valid
    sc = np.where(cond[:, None], -np.inf, sc)
    sc = sc - np.nanmax(np.where(np.isinf(sc), np.nan, sc), axis=-1,
                        keepdims=True)
    e = np.exp(sc)
    e = np.where(np.isnan(e), 0.0, e)
    att = e / np.maximum(e.sum(-1, keepdims=True), 1e-30)
    ho = np.einsum('bhst,bhte->bhse', att, v)
    cat = np.transpose(ho, (0, 2, 1, 3)).reshape(Bn, Sn, -1)
    mh = cat @ wo_w.T + wo_b
    x1 = ln(x + mh, ln1_g, ln1_b)
    hh = np.maximum(x1 @ ff1_w.T + ff1_b, 0.0)
    ff = hh @ ff2_w.T + ff2_b
    return ln(x1 + ff, ln2_g, ln2_b).astype(np.float32)


def _prep_inputs(inputs):
    bf = ml_dtypes.bfloat16
    x = np.asarray(inputs["x"], np.float32)
    wqT = np.ascontiguousarray(
        np.asarray(inputs["wq"], np.float32).transpose(2, 0, 1).reshape(D, HE)
    ).astype(bf).reshape(ND, 128, HE)
    wkT = np.ascontiguousarray(
        np.asarray(inputs["wk"], np.float32).transpose(2, 0, 1).reshape(D, HE)
    ).astype(bf).reshape(ND, 128, HE)
    wvT = np.ascontiguousarray(
        np.asarray(inputs["wv"], np.float32).transpose(2, 0, 1).reshape(D, HE)
    ).astype(bf).reshape(ND, 128, HE)
    woT = np.ascontiguousarray(np.asarray(inputs["wo_w"], np.float32).T
                               ).astype(bf).reshape(NT, 128, D)
    ff1T = np.ascontiguousarray(np.asarray(inputs["ff1_w"], np.float32).T
                                ).astype(bf).reshape(ND, 128, FF)
    ff2T = np.ascontiguousarray(np.asarray(inputs["ff2_w"], np.float32).T
                                ).astype(bf).reshape(NF, 128, D)
    shared = dict(
        wqT=wqT, wkT=wkT, wvT=wvT, woT=woT, ff1T=ff1T, ff2T=ff2T,
        wo_b=np.asarray(inputs["wo_b"], np.float32).reshape(ND, 128, 1),
        ff1_b=np.asarray(inputs["ff1_b"], np.float32).reshape(NF, 128, 1),
        ff2_b=np.asarray(inputs["ff2_b"], np.float32).reshape(ND, 128, 1),
        ln1_g=np.asarray(inputs["ln1_g"], np.float32).reshape(ND, 128, 1),
        ln1_b=np.asarray(inputs["ln1_b"], np.float32).reshape(ND, 128, 1),
        ln2_g=np.asarray(inputs["ln2_g"], np.float32).reshape(ND, 128, 1),
        ln2_b=np.asarray(inputs["ln2_b"], np.float32).reshape(ND, 128, 1),
    )
    in_maps = []
    for b in range(B):
        xT = np.ascontiguousarray(x[b].T)  # [D, S]
        m = dict(shared)
        m["xT"] = xT.reshape(ND, 128, S)
        m["xTb"] = xT.astype(bf).reshape(ND, 128, S)
        in_maps.append(m)
    return in_maps


def run_sharded(inputs, trace=False, trace_kwargs=None):
    if "nc" not in _CACHE:
        _CACHE["nc"] = _build()
    nc = _CACHE["nc"]
    in_maps = _prep_inputs(inputs)
    res = run_bass_kernel_spmd(nc, in_maps, list(range(N_CORES)), trace=trace,
                               **(trace_kwargs or {}))
    outs = []
    for b in range(B):
        r = np.asarray(res.results[b]["out"], np.float32).reshape(D, S)
        outs.append(r.T)
    return np.stack(outs), res


def kernel(**inputs) -> np.ndarray:
    mask = np.asarray(inputs["attention_mask"])
    if not np.all(mask != 0):
        return _np_reference(**{k: np.asarray(v) for k, v in inputs.items()})
    out, _ = run_sharded(inputs, trace=False)
    return out


# revision 39
# speedup vs baseline: 1.0163x; 1.0163x over previous
"""Trainium2 Bass kernel for nn_DecoderBlock (B=8, S=1024, D=256, H=4 heads
of full width 256, FF=1024).

Strategy: pure data parallelism — B=8 batch elements across 8 NeuronCores,
zero collectives. Per core, one full decoder block in "transposed" activation
layout (features on SBUF partitions, tokens on the free dim) so every matmul
chains without transposes:

  qT/kT = (wT)^T @ xT       (per head, [E, S])
  v     = (xT)^T @ wvT      (natural [S, E] — the att@v stationary operand)
  expT  = exp((kT^T qT)/16) (causal lower-triangle only, [t, s] tiles)
  Z     = ones^T @ expT     (softmax denominators; the ones[128,128] operand
                             leaves Z replicated on every partition, so the
                             reciprocal and normalize run partition-parallel)
  oT    = v^T @ expT        (unnormalized head outputs, normalized by 1/Z)
  mhT   = woT^T @ oT ; LN1 ; ffn (ff1 relu ff2) ; LN2   (all transposed)

LayerNorm means/mean-squares come from ones(1/D) matmuls, replicated across
partitions the same way. Weights are pre-transposed and cast to bf16 on the
host; matmuls run bf16 (4x the fp32 PE rate), accumulation and LN math in
fp32. Dummy warmup matmuls at kernel start keep the PE HAM clock-gate at
full rate while the input DMAs land.

The attention_mask input is all ones per the problem spec (causal mask only);
if a mask with zeros ever shows up, we fall back to a numpy reference.
"""

import numpy as np
import ml_dtypes

import concourse.bass as bass
import concourse.mybir as mybir
import concourse.tile as tile
from concourse import bacc
from concourse.bass_utils import run_bass_kernel_spmd

F32 = mybir.dt.float32
BF16 = mybir.dt.bfloat16
AF = mybir.ActivationFunctionType
ALU = mybir.AluOpType

N_CORES = 8
B, S, D, H, E, HE, FF = 8, 1024, 256, 4, 256, 1024, 1024
SC = 512          # token (free-dim) chunk
NJ = S // SC      # 2 chunks of tokens
ND = D // 128     # 2 partition chunks of features
NF = FF // 128    # 8 partition chunks of ff features
NT = S // 128     # 8 partition chunks of tokens
LN_EPS = 1e-5
SCALE = 1.0 / 16.0  # 1/sqrt(D)

_CACHE = {}


def _build():
    nc = bacc.Bacc("TRN2", target_bir_lowering=False, debug=False,
                   num_devices=N_CORES)

    # ---- DRAM parameters (per-core shard + replicated weights) ----
    xT_d = nc.dram_tensor("xT", [ND, 128, S], F32, kind="ExternalInput")
    xTb_d = nc.dram_tensor("xTb", [ND, 128, S], BF16, kind="ExternalInput")
    wqT_d = nc.dram_tensor("wqT", [ND, 128, HE], BF16, kind="ExternalInput")
    wkT_d = nc.dram_tensor("wkT", [ND, 128, HE], BF16, kind="ExternalInput")
    wvT_d = nc.dram_tensor("wvT", [ND, 128, HE], BF16, kind="ExternalInput")
    woT_d = nc.dram_tensor("woT", [NT, 128, D], BF16, kind="ExternalInput")
    ff1T_d = nc.dram_tensor("ff1T", [ND, 128, FF], BF16, kind="ExternalInput")
    ff2T_d = nc.dram_tensor("ff2T", [NF, 128, D], BF16, kind="ExternalInput")
    wo_b_d = nc.dram_tensor("wo_b", [ND, 128, 1], F32, kind="ExternalInput")
    ff1_b_d = nc.dram_tensor("ff1_b", [NF, 128, 1], F32, kind="ExternalInput")
    ff2_b_d = nc.dram_tensor("ff2_b", [ND, 128, 1], F32, kind="ExternalInput")
    ln1_g_d = nc.dram_tensor("ln1_g", [ND, 128, 1], F32, kind="ExternalInput")
    ln1_b_d = nc.dram_tensor("ln1_b", [ND, 128, 1], F32, kind="ExternalInput")
    ln2_g_d = nc.dram_tensor("ln2_g", [ND, 128, 1], F32, kind="ExternalInput")
    ln2_b_d = nc.dram_tensor("ln2_b", [ND, 128, 1], F32, kind="ExternalInput")
    out_d = nc.dram_tensor("out", [ND, 128, S], F32, kind="ExternalOutput")

    with tile.TileContext(nc) as tc:
        with tc.tile_pool(name="consts", bufs=1) as consts, \
             tc.tile_pool(name="acts", bufs=1) as acts, \
             tc.tile_pool(name="work", bufs=2) as work, \
             tc.tile_pool(name="lnp", bufs=1) as lnp, \
             tc.tile_pool(name="psA", bufs=3, space="PSUM") as psA, \
             tc.tile_pool(name="psO", bufs=3, space="PSUM") as psO, \
             tc.tile_pool(name="psZ", bufs=2, space="PSUM") as psZ:

            # ---- constants / weights into SBUF ----
            dma_engines = [nc.sync]
            dma_rr = [0]

            def load2(dram, shape, dt, n):
                ts_ = []
                for i in range(n):
                    t = consts.tile(shape, dt, tag=f"{dram.name}{i}", name=f"{dram.name}{i}")
                    eng = dma_engines[dma_rr[0] % len(dma_engines)]
                    dma_rr[0] += 1
                    eng.dma_start(out=t[:], in_=dram[i])
                    ts_.append(t)
                return ts_

            # DMA in first-use order so the PE can start ASAP
            xTb = load2(xTb_d, [128, S], BF16, ND)
            wqT = load2(wqT_d, [128, HE], BF16, ND)
            wkT = load2(wkT_d, [128, HE], BF16, ND)
            wvT = load2(wvT_d, [128, HE], BF16, ND)
            xT = load2(xT_d, [128, S], F32, ND)
            woT = load2(woT_d, [128, D], BF16, NT)
            ff1T = load2(ff1T_d, [128, FF], BF16, ND)
            ff2T = load2(ff2T_d, [128, D], BF16, NF)
            wo_b = load2(wo_b_d, [128, 1], F32, ND)
            ff1_b = load2(ff1_b_d, [128, 1], F32, NF)
            ff2_b = load2(ff2_b_d, [128, 1], F32, ND)
            ln1_g = load2(ln1_g_d, [128, 1], F32, ND)
            ln1_b = load2(ln1_b_d, [128, 1], F32, ND)
            ln2_g = load2(ln2_g_d, [128, 1], F32, ND)
            ln2_b = load2(ln2_b_d, [128, 1], F32, ND)

            # "ones" matrices for partition-dim reductions via matmul; the
            # [128,128] shape leaves the sum REPLICATED on all partitions so
            # downstream row math runs partition-parallel with no broadcast.
            ones_bf = consts.tile([128, 128], BF16, tag="ones", name="ones")
            nc.vector.memset(ones_bf[:], 1.0)
            invd_bf = consts.tile([128, 128], BF16, tag="invd", name="invd")
            nc.vector.memset(invd_bf[:], 1.0 / D)  # 2^-8, exact in bf16
            eps_t = consts.tile([128, 1], F32, tag="eps", name="eps")
            nc.vector.memset(eps_t[:], LN_EPS)
            # multiplicative causal mask for the diagonal 128x128 block of a
            # transposed [t, s] exp tile: 1 where t <= s else 0
            cmaskf = consts.tile([128, 128], F32, tag="cmaskf", name="cmaskf")
            nc.gpsimd.memset(cmaskf[:], 1.0)
            nc.gpsimd.affine_select(
                out=cmaskf[:], in_=cmaskf[:],
                compare_op=ALU.is_ge, fill=0.0,
                base=0, pattern=[[1, 128]], channel_multiplier=-1,
            )
            cmask = consts.tile([128, 128], BF16, tag="cmask", name="cmask")
            nc.vector.tensor_copy(out=cmask[:], in_=cmaskf[:])

            # PE warmup: ~4us of dummy matmuls (no DMA dependency) so the
            # HAM clock gate reaches 8/8 while the input DMAs land; real
            # matmuls then start at full clock.
            for wi in range(32):
                wp = psA.tile([128, 128], F32, tag="mm", name="warm")
                nc.tensor.matmul(wp[:], ones_bf[:], ones_bf[:],
                                 start=True, stop=True)

            # ---- attention: per head -> ONT [HE, S] normalized heads (bf16)
            ont = [acts.tile([128, S], BF16, tag=f"ont{c}", name=f"ont{c}") for c in range(NT)]

            for h in range(H):
                # Q^T, K^T [E, S] (transposed), V [S, E] (natural), bf16
                qh = [work.tile([128, S], BF16, tag=f"qh{e0}", name=f"qh{e0}") for e0 in range(2)]
                kh = [work.tile([128, S], BF16, tag=f"kh{e0}", name=f"kh{e0}") for e0 in range(2)]
                vh = [work.tile([128, E], BF16, tag=f"vh{t0}", name=f"vh{t0}") for t0 in range(NT)]
                for e0 in range(2):
                    for j in range(NJ):
                        cols = slice(j * SC, (j + 1) * SC)
                        qp = psA.tile([128, SC], F32, tag="mm", name="mm")
                        for d0 in range(ND):
                            nc.tensor.matmul(
                                qp[:], wqT[d0][:, h * E + e0 * 128: h * E + (e0 + 1) * 128],
                                xTb[d0][:, cols], start=(d0 == 0), stop=(d0 == ND - 1))
                        nc.scalar.copy(out=qh[e0][:, cols], in_=qp[:])
                        kp = psA.tile([128, SC], F32, tag="mm", name="mm")
                        for d0 in range(ND):
                            nc.tensor.matmul(
                                kp[:], wkT[d0][:, h * E + e0 * 128: h * E + (e0 + 1) * 128],
                                xTb[d0][:, cols], start=(d0 == 0), stop=(d0 == ND - 1))
                        nc.scalar.copy(out=kh[e0][:, cols], in_=kp[:])
                for t0 in range(NT):
                    vp = psA.tile([128, E], F32, tag="mm", name="mm")
                    for d0 in range(ND):
                        nc.tensor.matmul(
                            vp[:], xTb[d0][:, t0 * 128:(t0 + 1) * 128],
                            wvT[d0][:, h * E:(h + 1) * E],
                            start=(d0 == 0), stop=(d0 == ND - 1))
                    nc.vector.tensor_copy(out=vh[t0][:], in_=vp[:])

                zp = [psZ.tile([128, SC], F32, tag="z", name="z")
                      for j in range(NJ)]
                op = [[psO.tile([128, SC], F32, tag="o", name="o")
                       for _ in range(2)] for j in range(NJ)]
                for j in range(NJ):
                    kmax = 4 * j + 4
                    pend = []

                    def emit_zo(item):
                        jj, kk, ek, off, w = item
                        km = 4 * jj + 4
                        nc.tensor.matmul(
                            zp[jj][:, off:off + w], ones_bf[:],
                            ek[:, off:off + w],
                            start=(kk == 0), stop=(kk == km - 1),
                            skip_group_check=True)
                        for e0 in range(2):
                            nc.tensor.matmul(
                                op[jj][e0][:, off:off + w],
                                vh[kk][:, e0 * 128:(e0 + 1) * 128],
                                ek[:, off:off + w],
                                start=(kk == 0), stop=(kk == km - 1),
                                skip_group_check=True)

                    for k in range(kmax):
                        start_col = max(SC * j, 128 * k)
                        off = start_col - SC * j
                        w = SC - off
                        sp = psA.tile([128, SC], F32, tag="mm", name="mm")
                        for e0 in range(2):
                            nc.tensor.matmul(
                                sp[:, off:off + w],
                                kh[e0][:, 128 * k:128 * (k + 1)],
                                qh[e0][:, start_col:start_col + w],
                                start=(e0 == 0), stop=(e0 == 1))
                        ek = work.tile([128, SC], BF16, tag=f"exp{k}",
                                       name=f"exp{k}")
                        nc.scalar.activation(
                            out=ek[:, off:off + w], in_=sp[:, off:off + w],
                            func=AF.Exp, scale=SCALE)
                        if 128 * k >= SC * j:  # diagonal band: causal mask
                            nc.vector.tensor_mul(
                                out=ek[:, off:off + 128],
                                in0=ek[:, off:off + 128], in1=cmask[:])
                        pend.append((j, k, ek, off, w))
                        if len(pend) > 2:
                            emit_zo(pend.pop(0))
                    for item in pend:
                        emit_zo(item)
                for j in range(NJ):
                    # normalize: ONT = oT * (1/Z); Z is replicated on all
                    # partitions by the ones[128,128] matmul
                    zb = work.tile([128, SC], F32, tag="zb", name="zb")
                    nc.vector.reciprocal_approx_fast(out=zb[:], in_=zp[j][:])
                    cols = slice(j * SC, (j + 1) * SC)
                    for e0 in range(2):
                        nc.vector.tensor_mul(
                            out=ont[h * 2 + e0][:, cols], in0=op[j][e0][:],
                            in1=zb[:])

            # ---- wo projection + residual -> R1 (f32, transposed) ----
            r1 = [acts.tile([128, S], F32, tag=f"r1_{d0}", name=f"r1_{d0}") for d0 in range(ND)]
            r1b = [lnp.tile([128, S], BF16, tag=f"lnsrcb{d0}", name=f"r1b_{d0}") for d0 in range(ND)]
            for d0 in range(ND):
                for j in range(NJ):
                    cols = slice(j * SC, (j + 1) * SC)
                    pp = psA.tile([128, SC], F32, tag="mm", name="mm")
                    for c in range(NT):
                        nc.tensor.matmul(
                            pp[:], woT[c][:, d0 * 128:(d0 + 1) * 128],
                            ont[c][:, cols], start=(c == 0), stop=(c == NT - 1))
                    # R1 = psum + wo_b + x (f32 + bf16 twin for LN stats)
                    nc.vector.scalar_tensor_tensor(
                        out=r1[d0][:, cols], in0=pp[:], scalar=wo_b[d0][:],
                        in1=xT[d0][:, cols], op0=ALU.add, op1=ALU.add)
                    nc.vector.scalar_tensor_tensor(
                        out=r1b[d0][:, cols], in0=pp[:], scalar=wo_b[d0][:],
                        in1=xT[d0][:, cols], op0=ALU.add, op1=ALU.add)

            def layernorm(src, srcb, gamma, beta, dst, tag, dma_out=None,
                          dst_bf=None):
                """src/srcb: ND x [128, S] f32/bf16 tiles (features on
                partitions). dst: ND x [128, S] f32; dst_bf: optional bf16
                copies; dma_out: optional DRAM targets."""
                sq = [lnp.tile([128, S], BF16, tag=f"lnsq{d0}", name=f"{tag}sq{d0}")
                      for d0 in range(ND)]
                for d0 in range(ND):
                    nc.vector.tensor_mul(out=sq[d0][:], in0=srcb[d0][:],
                                         in1=srcb[d0][:])
                for j in range(NJ):
                    cols = slice(j * SC, (j + 1) * SC)
                    # mean / mean-of-squares directly (ones are 1/D),
                    # replicated on all 128 partitions
                    mup = psZ.tile([128, SC], F32, tag="z", name="z")
                    for d0 in range(ND):
                        nc.tensor.matmul(mup[:], invd_bf[:],
                                         srcb[d0][:, cols],
                                         start=(d0 == 0), stop=(d0 == ND - 1))
                    m2p = psZ.tile([128, SC], F32, tag="z", name="z")
                    for d0 in range(ND):
                        nc.tensor.matmul(m2p[:], invd_bf[:],
                                         sq[d0][:, cols],
                                         start=(d0 == 0), stop=(d0 == ND - 1))
                    musq = work.tile([128, SC], F32, tag="musq", name="musq")
                    nc.scalar.activation(out=musq[:], in_=mup[:],
                                         func=AF.Square)
                    var = work.tile([128, SC], F32, tag="var", name="var")
                    nc.vector.tensor_sub(out=var[:], in0=m2p[:], in1=musq[:])
                    sd = work.tile([128, SC], F32, tag="sd", name="sd")
                    nc.scalar.activation(out=sd[:], in_=var[:], func=AF.Sqrt,
                                         bias=eps_t[:])
                    rstd = work.tile([128, SC], F32, tag="rstd", name="rstd")
                    nc.vector.reciprocal_approx_fast(out=rstd[:], in_=sd[:])
                    mr = work.tile([128, SC], F32, tag="mr", name="mr")
                    nc.vector.tensor_mul(out=mr[:], in0=mup[:], in1=rstd[:])
                    for d0 in range(ND):
                        t = work.tile([128, SC], F32, tag="lnt", name="lnt")
                        nc.vector.tensor_mul(out=t[:], in0=src[d0][:, cols],
                                             in1=rstd[:])
                        nc.vector.tensor_sub(out=t[:], in0=t[:], in1=mr[:])
                        nc.vector.tensor_scalar(
                            out=dst[d0][:, cols], in0=t[:],
                            scalar1=gamma[d0][:], scalar2=beta[d0][:],
                            op0=ALU.mult, op1=ALU.add)
                        if dst_bf is not None:
                            nc.scalar.copy(out=dst_bf[d0][:, cols],
                                           in_=dst[d0][:, cols])
                        if dma_out is not None:
                            nc.sync.dma_start(out=dma_out[d0][:, cols],
                                              in_=dst[d0][:, cols])

            # ---- LN1 -> x1 (f32) + bf16 copy for the FFN ----
            x1 = [acts.tile([128, S], F32, tag=f"x1_{d0}", name=f"x1_{d0}") for d0 in range(ND)]
            x1b = [acts.tile([128, S], BF16, tag=f"x1b{d0}", name=f"x1b{d0}") for d0 in range(ND)]
            layernorm(r1, r1b, ln1_g, ln1_b, x1, "ln1", dst_bf=x1b)

            # ---- FFN ----
            hT = [acts.tile([128, S], BF16, tag=f"hT{f0}", name=f"hT{f0}") for f0 in range(NF)]
            for f0 in range(NF):
                for j in range(NJ):
                    cols = slice(j * SC, (j + 1) * SC)
                    fp = psA.tile([128, SC], F32, tag="mm", name="mm")
                    for d0 in range(ND):
                        nc.tensor.matmul(
                            fp[:], ff1T[d0][:, f0 * 128:(f0 + 1) * 128],
                            x1b[d0][:, cols], start=(d0 == 0), stop=(d0 == ND - 1))
                    # relu(psum + b) fused on ACT
                    nc.scalar.activation(out=hT[f0][:, cols], in_=fp[:],
                                         func=AF.Relu, bias=ff1_b[f0][:])
            r2 = [acts.tile([128, S], F32, tag=f"r2_{d0}", name=f"r2_{d0}") for d0 in range(ND)]
            r2b = [lnp.tile([128, S], BF16, tag=f"lnsrcb{d0}", name=f"r2b_{d0}") for d0 in range(ND)]
            for d0 in range(ND):
                for j in range(NJ):
                    cols = slice(j * SC, (j + 1) * SC)
                    fp = psA.tile([128, SC], F32, tag="mm", name="mm")
                    for f0 in range(NF):
                        nc.tensor.matmul(
                            fp[:], ff2T[f0][:, d0 * 128:(d0 + 1) * 128],
                            hT[f0][:, cols], start=(f0 == 0), stop=(f0 == NF - 1))
                    nc.vector.scalar_tensor_tensor(
                        out=r2[d0][:, cols], in0=fp[:], scalar=ff2_b[d0][:],
                        in1=x1[d0][:, cols], op0=ALU.add, op1=ALU.add)
                    nc.vector.scalar_tensor_tensor(
                        out=r2b[d0][:, cols], in0=fp[:], scalar=ff2_b[d0][:],
                        in1=x1[d0][:, cols], op0=ALU.add, op1=ALU.add)

            # ---- LN2 -> out ----
            outT = [acts.tile([128, S], F32, tag=f"out{d0}", name=f"out{d0}") for d0 in range(ND)]
            layernorm(r2, r2b, ln2_g, ln2_b, outT, "ln2",
                      dma_out=[out_d[d0] for d0 in range(ND)])

    nc.compile()
    return nc


def _np_reference(x, attention_mask, wq, wk, wv, wo_w, wo_b, ln1_g, ln1_b,
                  ff1_w, ff1_b, ff2_w, ff2_b, ln2_g, ln2_b):
    """Numpy fallback (only used if attention_mask has zeros)."""
    def ln(t, g, b):
        mu = t.mean(-1, keepdims=True)
        var = t.var(-1, keepdims=True)
        return (t - mu) / np.sqrt(var + LN_EPS) * g + b
    Bn, Sn, Dn = x.shape
    q = np.einsum('bsd,hed->bhse', x, wq)
    k = np.einsum('bsd,hed->bhse', x, wk)
    v = np.einsum('bsd,hed->bhse', x, wv)
    sc = np.einsum('bhse,bhte->bhst', q, k) / np.sqrt(np.float32(Dn))
    idx = np.arange(Sn)
    causal = idx[None, :] > idx[:, None]
    m = attention_mask.astype(bool)
    valid = m[:, None, :] & m[:, :, None]
    cond = causal[None] | ~valid
    sc = np.where(cond[:, None], -np.inf, sc)
    sc = sc - np.nanmax(np.where(np.isinf(sc), np.nan, sc), axis=-1,
                        keepdims=True)
    e = np.exp(sc)
    e = np.where(np.isnan(e), 0.0, e)
    att = e / np.maximum(e.sum(-1, keepdims=True), 1e-30)
    ho = np.einsum('bhst,bhte->bhse', att, v)
    cat = np.transpose(ho, (0, 2, 1, 3)).reshape(Bn, Sn, -1)
    mh = cat @ wo_w.T + wo_b
    x1 = ln(x + mh, ln1_g, ln1_b)
    hh = np.maximum(x1 @ ff1_w.T + ff1_b, 0.0)
    ff = hh @ ff2_w.T + ff2_b
    return ln(x1 + ff, ln2_g, ln2_b).astype(np.float32)


def _prep_inputs(inputs):
    bf = ml_dtypes.bfloat16
    x = np.asarray(inputs["x"], np.float32)
    wqT = np.ascontiguousarray(
        np.asarray(inputs["wq"], np.float32).transpose(2, 0, 1).reshape(D, HE)
    ).astype(bf).reshape(ND, 128, HE)
    wkT = np.ascontiguousarray(
        np.asarray(inputs["wk"], np.float32).transpose(2, 0, 1).reshape(D, HE)
    ).astype(bf).reshape(ND, 128, HE)
    wvT = np.ascontiguousarray(
        np.asarray(inputs["wv"], np.float32).transpose(2, 0, 1).reshape(D, HE)
    ).astype(bf).reshape(ND, 128, HE)
    woT = np.ascontiguousarray(np.asarray(inputs["wo_w"], np.float32).T
                               ).astype(bf).reshape(NT, 128, D)
    ff1T = np.ascontiguousarray(np.asarray(inputs["ff1_w"], np.float32).T
                                ).astype(bf).reshape(ND, 128, FF)
    ff2T = np.ascontiguousarray(np.asarray(inputs["ff2_w"], np.float32).T
                                ).astype(bf).reshape(NF, 128, D)
    shared = dict(
        wqT=wqT, wkT=wkT, wvT=wvT, woT=woT, ff1T=ff1T, ff2T=ff2T,
        wo_b=np.asarray(inputs["wo_b"], np.float32).reshape(ND, 128, 1),
        ff1_b=np.asarray(inputs["ff1_b"], np.float32).reshape(NF, 128, 1),
        ff2_b=np.asarray(inputs["ff2_b"], np.float32).reshape(ND, 128, 1),
        ln1_g=np.asarray(inputs["ln1_g"], np.float32).reshape(ND, 128, 1),
        ln1_b=np.asarray(inputs["ln1_b"], np.float32).reshape(ND, 128, 1),
        ln2_g=np.asarray(inputs["ln2_g"], np.float32).reshape(ND, 128, 1),
        ln2_b=np.asarray(inputs["ln2_b"], np.float32).reshape(ND, 128, 1),
    )
    in_maps = []
    for b in range(B):
        xT = np.ascontiguousarray(x[b].T)  # [D, S]
        m = dict(shared)
        m["xT"] = xT.reshape(ND, 128, S)
        m["xTb"] = xT.astype(bf).reshape(ND, 128, S)
        in_maps.append(m)
    return in_maps


def run_sharded(inputs, trace=False, trace_kwargs=None):
    if "nc" not in _CACHE:
        _CACHE["nc"] = _build()
    nc = _CACHE["nc"]
    in_maps = _prep_inputs(inputs)
    res = run_bass_kernel_spmd(nc, in_maps, list(range(N_CORES)), trace=trace,
                               **(trace_kwargs or {}))
    outs = []
    for b in range(B):
        r = np.asarray(res.results[b]["out"], np.float32).reshape(D, S)
        outs.append(r.T)
    return np.stack(outs), res


def kernel(**inputs) -> np.ndarray:
    mask = np.asarray(inputs["attention_mask"])
    if not np.all(mask != 0):
        return _np_reference(**{k: np.asarray(v) for k, v in inputs.items()})
    out, _ = run_sharded(inputs, trace=False)
    return out
